# revision 11
# baseline (speedup 1.0000x reference)
"""ContTimeLSTM Trainium2 kernel.

B=64, L=2048, DIN=256, H=256. Data-parallel over batch: 8 cores x 8 rows.
Per core:
  - x-projection (px = x @ Wx.T + bias) computed on-device in windowed
    batched fp32r matmuls, staged in SBUF as fp16
  - per-step recurrent matmul h @ Wh.T in fp16 (28 [128,128]x[128,8] tiles
    accumulating onto an identity-matmul px preload in PSUM)
  - all gates via one resident ACT table set (exp_and_others):
    sigmoid = 0.5 + 0.5*tanh(x/2) (z-weights pre-doubled so tanh serves z),
    softplus = relu(x) + poly(exp(-|x|)) via a custom fused DVE op
Layouts: partition dim = h-within-128-chunk, free = (h-chunk, batch).

Self-contained: hardcodes shapes/sharding.
"""

import numpy as np
import ml_dtypes

import concourse.bacc as bacc
import concourse.mybir as mybir
from concourse.tile import TileContext
import concourse.bass as bass

B, L_FULL, DIN, H = 64, 2048, 256, 256
NCORES = 8
BLOC = B // NCORES          # 8 batch rows per core
HL = 128                    # partition dim (h within chunk)
HO = 2                      # h chunks
NM = 14                     # 7H/128 output chunks
NK = 2                      # contraction chunks (DIN or H = 256)
FREE = HO * BLOC            # 16
# our gate order: f, i, ie, fe, o, z, d   (ref: i, f, ie, fe, o, z, d)
GATE_SRC = [1, 0, 2, 3, 4, 5, 6]   # ours[g] = ref gate GATE_SRC[g]

# ln(1+m) ~= m + m^2*(c2 + c3*m + c4*m^2) on [0,1], max abs err 1.5e-4
PC2, PC3, PC4 = -0.4851556543060101, 0.24802679623138768, -0.06987100775791244

AF = mybir.ActivationFunctionType
OP = mybir.AluOpType
F32, F32R, F16 = mybir.dt.float32, mybir.dt.float32r, mybir.dt.float16

_SPOLY = None


def _register_spoly():
    """Custom DVE op: out = poly(Src0) + Src1; Src0=exp(-|pd|), Src1=relu(pd)."""
    global _SPOLY
    if _SPOLY is not None:
        return _SPOLY
    from concourse.dve_spec import Spec, Src0, Src1, C0, C1, C2, lower
    from concourse.dve_ops import DveOp
    import concourse.dve_ops as _do
    from concourse.dve_uop import DveOpSpec

    m2 = Src0 * Src0
    a = Src0 * C1 + C0
    b = m2 * C2 + a
    body = (Src0 + m2 * b) + Src1
    op = DveOp(
        "CTL_SOFTPLUS",
        Spec(
            body=body,
            reference=lambda in0, in1, s0, s1, imm2: in0
            + in0 * in0 * (s0 + s1 * in0 + imm2 * in0 * in0)
            + in1,
        ),
        subdim=False,
        uops_sha={},
    )
    for ver in ("v3", "v4"):
        sp = DveOpSpec(name=op.name, opcode=0, uops=lower(op.spec, ver=ver),
                       rd1_en=True)
        op.uops_sha[ver] = sp.sha(ver)
    _do.OPS.append(op)
    _do.CUSTOM_DVE_SPECS[op.name] = op.spec
    _do._SUB_OPCODE_FOR_NAME[op.name] = _do._CUSTOM_DVE_ROW_BASE + len(_do.OPS) - 1
    _SPOLY = op
    return op


def build_nc(L, wsteps=64, body_windows=2, use_for_i=True):
    """Build the per-core Bass program (SPMD; same NEFF on all 8 cores)."""
    spoly = _register_spoly()
    nwin = L // wsteps
    NW = wsteps * BLOC  # px free size per window (512 at wsteps=64)
    LPAD = L + wsteps   # x/ndt padded by one window for harmless prefetch

    nc = bacc.Bacc("TRN2", target_bir_lowering=False, debug=False,
                   num_devices=NCORES)

    d_x = nc.dram_tensor("x_in", [NK, HL, LPAD, BLOC], F32R, kind="ExternalInput")
    d_wh = nc.dram_tensor("wh_in", [HL, NM * NK, HL], F16, kind="ExternalInput")
    d_wx = nc.dram_tensor("wx_in", [HL, NM * NK, HL], F32R, kind="ExternalInput")
    d_bias = nc.dram_tensor("bias_in", [1, NM, HL], F32R, kind="ExternalInput")
    d_ident = nc.dram_tensor("ident_in", [HL, HL], F16, kind="ExternalInput")
    d_ones = nc.dram_tensor("ones_in", [1, NW], F32R, kind="ExternalInput")
    d_ndt = nc.dram_tensor("ndt_in", [LPAD, FREE], F32, kind="ExternalInput")
    d_ndt0 = nc.dram_tensor("ndt0_in", [1, FREE], F32, kind="ExternalInput")
    d_x0 = nc.dram_tensor("x0_in", [HL, NK * 2], F32R, kind="ExternalInput")
    d_yh = nc.dram_tensor("yh", [HL, L, FREE], F32, kind="ExternalOutput")
    d_ys = nc.dram_tensor("ys", [HL, L, 4, FREE], F32, kind="ExternalOutput")

    with TileContext(nc) as tc:
        with (
            tc.tile_pool(name="res", bufs=1) as res,
            tc.tile_pool(name="stp", bufs=3) as stp,
            tc.tile_pool(name="ps", bufs=4, space="PSUM") as ps,
            tc.tile_pool(name="pxps", bufs=2, space="PSUM") as pxps,
        ):
            # resident tiles
            wh = res.tile([HL, NM * NK, HL], F16, tag="wh")
            wx = res.tile([HL, NM * NK, HL], F32R, tag="wx")
            bias = res.tile([1, NM, HL], F32R, tag="bias")
            ident = res.tile([HL, HL], F16, tag="ident")
            ones = res.tile([1, NW], F32R, tag="ones")
            halfs = res.tile([HL, 80], F32, tag="halfs")
            x0 = res.tile([HL, NK * 2], F32R, tag="x0")
            ndt0b = res.tile([HL, FREE], F32, tag="ndt0b")
            nc.sync.dma_start(wh[:], d_wh.ap()[:])
            nc.sync.dma_start(wx[:], d_wx.ap()[:])
            nc.sync.dma_start(bias[:], d_bias.ap()[:])
            nc.sync.dma_start(ident[:], d_ident.ap()[:])
            nc.sync.dma_start(ones[:], d_ones.ap()[:])
            nc.sync.dma_start(x0[:], d_x0.ap()[:])
            nc.sync.dma_start(ndt0b[:], d_ndt0.ap()[:].partition_broadcast(HL))
            nc.vector.memset(halfs[:], 0.5)

            # double-buffered window IO + rings
            xwin = res.tile([HL, 2, NK, NW], F32R, tag="xwin")
            ndtb = res.tile([HL, 2, wsteps, FREE], F32, tag="ndtb")
            pxbuf = res.tile([HL, 2, NM, NW], F16, tag="pxbuf")
            hring = res.tile([HL, 2, wsteps, FREE], F32, tag="hring")
            sring = res.tile([HL, 2, wsteps, 4, FREE], F32, tag="sring")
            # rings of 4 (wsteps % 4 == 0 keeps indices window-static)
            vring = res.tile([HL, 4, 80], F32, tag="vring")
            ering = res.tile([HL, 4, FREE], F32, tag="ering")
            hfring = res.tile([HL, 4, HO, BLOC], F16, tag="hfring")

            def a2(ap):
                return ap.rearrange("p (a b) -> p a b", a=2)

            def px_phase(par, woff):
                """px for one window into pxbuf[par]; woff = t offset (steps)."""
                for k in range(NK):
                    nc.sync.dma_start(
                        xwin[:, par, k, :],
                        d_x.ap()[k, :, bass.ds(woff, wsteps), :])
                nc.sync.dma_start(
                    ndtb[:, par, :, :],
                    d_ndt.ap()[bass.ds(woff, wsteps), :].partition_broadcast(HL))
                for m in range(NM):
                    pt = pxps.tile([HL, NW], F32, tag="pxp")
                    for k in range(NK):
                        nc.tensor.matmul(pt[:], wx[:, m * NK + k, :],
                                         xwin[:, par, k, :],
                                         start=(k == 0), stop=False,
                                         skip_group_check=True)
                    nc.tensor.matmul(pt[:], bias[:, m, :], ones[:],
                                     start=False, stop=True,
                                     skip_group_check=True)
                    nc.scalar.copy(pxbuf[:, par, m, :], pt[:])

            def step(par, s):
                r4, r4n = s % 4, (s + 1) % 4
                sm1 = (s - 1) % wsteps
                parm1 = par if s > 0 else 1 - par
                V = vring[:, r4, :]
                Vn = vring[:, r4n, :]
                E = ering[:, r4, :]
                En = ering[:, r4n, :]
                o_prev = sring[:, parm1, sm1, 0, :]

                # c = cs + (ce - cs)*E ; ce = V[48:64], cs = V[64:80]
                u = stp.tile([HL, FREE], F32, tag="u")
                nc.vector.tensor_tensor(u[:], V[:, 48:64], V[:, 64:80],
                                        op=OP.subtract)
                t1 = stp.tile([HL, FREE], F32, tag="t1")
                nc.vector.tensor_tensor(t1[:], u[:], E[:], op=OP.mult)
                nc.vector.tensor_tensor(V[:, 0:16], V[:, 64:80], t1[:], op=OP.add)
                Tc = stp.tile([HL, FREE], F32, tag="Tc")
                nc.scalar.activation(Tc[:], V[:, 0:16], AF.Tanh)
                # h in fp16 for the PE (DVE) and fp32 for output (POOL)
                nc.vector.tensor_tensor(
                    hfring[:, r4, :, :].rearrange("p a b -> p (a b)"),
                    o_prev, Tc[:], op=OP.mult)
                nc.gpsimd.tensor_tensor(hring[:, par, s, :], o_prev, Tc[:],
                                        op=OP.mult)

                # PE: px preload + 28 fp16 matmuls (d and z chunks first)
                pm = ps.tile([HL, NM * BLOC], F32, tag="proj")
                nc.tensor.matmul(
                    pm[:].rearrange("p (m b) -> p m b", m=NM), ident[:],
                    pxbuf[:, par, :, s * BLOC:(s + 1) * BLOC],
                    start=True, stop=False, skip_group_check=True)
                morder = [12, 13, 10, 11] + list(range(10))
                for mi, m in enumerate(morder):
                    for k in range(NK):
                        last = (mi == len(morder) - 1) and (k == NK - 1)
                        nc.tensor.matmul(pm[:, m * BLOC:(m + 1) * BLOC],
                                         wh[:, m * NK + k, :],
                                         hfring[:, r4, k, :],
                                         start=False, stop=last,
                                         skip_group_check=True)

                # d path -> E_{s+1};  d_s output = softplus(pd) via custom op
                pd = pm[:, 12 * BLOC:14 * BLOC]
                A2 = stp.tile([HL, FREE], F32, tag="A2")
                nc.scalar.activation(A2[:], pd, AF.Abs, scale=0.5)
                mE = stp.tile([HL, FREE], F32, tag="mE")
                nc.scalar.activation(mE[:], A2[:], AF.Exp, scale=-2.0)
                Rr = stp.tile([HL, FREE], F32, tag="Rr")
                nc.vector.scalar_tensor_tensor(Rr[:], pd, 0.5, A2[:],
                                               op0=OP.mult, op1=OP.add)
                Ld = sring[:, par, s, 3, :]
                nc.vector._custom_dve(spoly, out=Ld, in0=mE[:], in1=Rr[:],
                                      s0=PC2, s1=PC3, imm2=PC4)
                aE = stp.tile([HL, FREE], F32, tag="aE")
                nc.vector.tensor_tensor(aE[:], Ld, ndtb[:, par, s, :],
                                        op=OP.mult)
                nc.scalar.activation(En[:], aE[:], AF.Exp)

                # z -> V[16:48] (dual write); sigma gates via tanh
                nc.scalar.activation(
                    a2(V[:, 16:48]),
                    pm[:, 10 * BLOC:12 * BLOC]
                    .rearrange("p (o f) -> p o f", o=1)
                    .broadcast_to([HL, 2, FREE]),
                    AF.Tanh, scale=0.5)
                T5 = stp.tile([HL, 80], F32, tag="T5")
                nc.scalar.activation(T5[:], pm[:, 0:10 * BLOC], AF.Tanh,
                                     scale=0.5)
                S4 = stp.tile([HL, 64], F32, tag="S4")
                nc.vector.scalar_tensor_tensor(S4[:], T5[:, 0:64], 0.5,
                                               halfs[:, 0:64],
                                               op0=OP.mult, op1=OP.add)
                nc.vector.scalar_tensor_tensor(sring[:, par, s, 0, :],
                                               T5[:, 64:80], 0.5,
                                               halfs[:, 0:16],
                                               op0=OP.mult, op1=OP.add)
                # P = [f,i,ie,fe]*[c,Z,Z,ce];  ce'=P2+P3 -> Vn[48:64],
                # cs'=P0+P1 -> Vn[64:80]
                P = stp.tile([HL, 64], F32, tag="P")
                nc.vector.tensor_tensor(P[:], S4[:], V[:, 0:64], op=OP.mult)
                P4 = P[:].rearrange("p (a b) -> p a b", a=4)
                nc.vector.tensor_tensor(a2(Vn[:, 48:80]),
                                        P4[:, 2::-2, :], P4[:, 3::-2, :],
                                        op=OP.add)
                # lazy (cs, ce) copy into state ring for output (POOL)
                nc.gpsimd.tensor_copy(sring[:, par, s, 1:3, :],
                                      a2(Vn[:, 48:80])[:, ::-1, :])

            def drain(par, woff):
                nc.sync.dma_start(d_yh.ap()[:, bass.ds(woff, wsteps), :],
                                  hring[:, par, :, :])
                nc.sync.dma_start(d_ys.ap()[:, bass.ds(woff, wsteps), :, :],
                                  sring[:, par, :, :, :])

            # ---- prologue: initial state from `start` projection ----
            p0 = pxps.tile([HL, NM * 2], F32, tag="pxp")
            for m in range(NM):
                for k in range(NK):
                    nc.tensor.matmul(p0[:, 2 * m:2 * m + 2],
                                     wx[:, m * NK + k, :],
                                     x0[:, 2 * k:2 * k + 2],
                                     start=(m == 0 and k == 0), stop=False,
                                     skip_group_check=True)
                nc.tensor.matmul(p0[:, 2 * m:2 * m + 2], bias[:, m, :],
                                 ones[:, 0:2], start=False,
                                 stop=(m == NM - 1), skip_group_check=True)
            p0m = p0[:].rearrange("p (m d) -> p m d", d=2)
            T50 = stp.tile([HL, 10], F32, tag="T50")
            nc.scalar.activation(T50[:], p0m[:, 0:10, 0:1], AF.Tanh, scale=0.5)
            z0 = stp.tile([HL, 2], F32, tag="z0")
            nc.scalar.activation(z0[:], p0m[:, 10:12, 0:1], AF.Tanh, scale=0.5)

            def bc2(src2):  # [HL,2] -> [HL,2,BLOC] broadcast read
                return src2.broadcast_to([HL, 2, BLOC])

            si0 = stp.tile([HL, FREE], F32, tag="si0")
            nc.vector.scalar_tensor_tensor(a2(si0[:]), bc2(T50[:, 2:4]), 0.5,
                                           a2(halfs[:, 0:16]),
                                           op0=OP.mult, op1=OP.add)
            sie0 = stp.tile([HL, FREE], F32, tag="sie0")
            nc.vector.scalar_tensor_tensor(a2(sie0[:]), bc2(T50[:, 4:6]), 0.5,
                                           a2(halfs[:, 0:16]),
                                           op0=OP.mult, op1=OP.add)
            nc.vector.scalar_tensor_tensor(a2(sring[:, 1, wsteps - 1, 0, :]),
                                           bc2(T50[:, 8:10]), 0.5,
                                           a2(halfs[:, 0:16]),
                                           op0=OP.mult, op1=OP.add)
            nc.vector.tensor_tensor(a2(vring[:, 0, 64:80]), a2(si0[:]),
                                    bc2(z0[:]), op=OP.mult)
            nc.vector.tensor_tensor(a2(vring[:, 0, 48:64]), a2(sie0[:]),
                                    bc2(z0[:]), op=OP.mult)
            A20 = stp.tile([HL, 2], F32, tag="A20")
            nc.scalar.activation(A20[:], p0m[:, 12:14, 0:1], AF.Abs, scale=0.5)
            m0 = stp.tile([HL, 2], F32, tag="m0")
            nc.scalar.activation(m0[:], A20[:], AF.Exp, scale=-2.0)
            R0 = stp.tile([HL, 2], F32, tag="R0")
            nc.vector.scalar_tensor_tensor(R0[:], p0m[:, 12:14, 0], 0.5, A20[:],
                                           op0=OP.mult, op1=OP.add)
            L0 = stp.tile([HL, 2], F32, tag="L0")
            nc.vector._custom_dve(spoly, out=L0[:], in0=m0[:], in1=R0[:],
                                  s0=PC2, s1=PC3, imm2=PC4)
            aE0 = stp.tile([HL, FREE], F32, tag="aE0")
            nc.vector.tensor_tensor(a2(aE0[:]), bc2(L0[:]), a2(ndt0b[:]),
                                    op=OP.mult)
            nc.scalar.activation(ering[:, 0, :], aE0[:], AF.Exp)

            # ---- main loop ----
            def window(par, woff):
                for s in range(wsteps):
                    step(par, s)
                drain(par, woff)

            px_phase(0, 0)
            if use_for_i:
                assert nwin % body_windows == 0
                nbody = nwin // body_windows
                bw = body_windows * wsteps
                with tc.For_i(0, nbody, 1) as i:
                    for j in range(body_windows):
                        par = j % 2
                        px_phase(1 - par, i * bw + (j + 1) * wsteps)
                        window(par, i * bw + j * wsteps)
            else:
                for w in range(nwin):
                    par = w % 2
                    px_phase(1 - par, (w + 1) * wsteps)
                    window(par, w * wsteps)

    nc.compile()
    return nc


# ---------------- host side ----------------

def _prep_inputs(x, time_deltas, start, weight, bias, L, wsteps):
    """Per-core input dicts. Pure layout/cast work."""
    LPAD = L + wsteps
    f16 = ml_dtypes.float16 if hasattr(ml_dtypes, "float16") else np.float16
    w = np.asarray(weight, np.float32)
    b = np.asarray(bias, np.float32)
    # permute gates to our order, double the z rows
    wp = np.concatenate([w[g * H:(g + 1) * H] for g in GATE_SRC], 0).copy()
    bp = np.concatenate([b[g * H:(g + 1) * H] for g in GATE_SRC], 0).copy()
    wp[5 * H:6 * H] *= 2.0
    bp[5 * H:6 * H] *= 2.0
    wx_, wh_ = wp[:, :DIN], wp[:, DIN:]
    # lhsT tiles: [r(=K within chunk), m*NK+k, c(=M within chunk)]
    def tiles(W):
        t = np.empty((HL, NM * NK, HL), np.float32)
        for m in range(NM):
            for k in range(NK):
                t[:, m * NK + k, :] = W[m * HL:(m + 1) * HL,
                                        k * HL:(k + 1) * HL].T
        return t
    wh_in = tiles(wh_).astype(np.float16)
    wx_in = tiles(wx_)
    bias_in = bp.reshape(1, NM, HL)
    ident_in = np.eye(HL, dtype=np.float16)
    ones_in = np.ones((1, wsteps * BLOC), np.float32)
    x0_in = np.repeat(np.asarray(start, np.float32).reshape(NK, HL).T, 2,
                      axis=1).copy()  # [HL, NK*2] duplicated columns

    ins = []
    for c in range(NCORES):
        bs = slice(c * BLOC, (c + 1) * BLOC)
        xc = np.asarray(x[bs], np.float32)           # [8, L, 256]
        # x_in[k, r, t, b] = x[b, t, k*128+r]
        x_in = np.zeros((NK, HL, LPAD, BLOC), np.float32)
        x_in[:, :, :L, :] = (xc.transpose(2, 1, 0)
                             .reshape(NK, HL, L, BLOC))
        dt = np.asarray(time_deltas[bs], np.float32)  # [8, L]
        ndt = np.zeros((LPAD, FREE), np.float32)
        # row t = -delta_{t+1}, broadcast over ho
        sh = -dt[:, 1:].T                            # [L-1, 8]
        ndt[:L - 1, 0:BLOC] = sh
        ndt[:L - 1, BLOC:FREE] = sh
        ndt0 = np.concatenate([-dt[:, 0], -dt[:, 0]]).reshape(1, FREE)
        ins.append(dict(x_in=x_in, wh_in=wh_in, wx_in=wx_in, bias_in=bias_in,
                        ident_in=ident_in, ones_in=ones_in, ndt_in=ndt,
                        ndt0_in=ndt0, x0_in=x0_in))
    return ins


def _gather(results, L):
    outs = np.empty((B, L, H), np.float32)
    states = np.empty((B, L, 4 * H), np.float32)
    for c, r in enumerate(results):
        bs = slice(c * BLOC, (c + 1) * BLOC)
        yh = r["yh"]                          # [128, L, 16]
        ys = r["ys"]                          # [128, L, 4, 16]
        outs[bs] = (yh.reshape(HL, L, HO, BLOC)
                    .transpose(3, 1, 2, 0).reshape(BLOC, L, H))
        states[bs] = (ys.reshape(HL, L, 4, HO, BLOC)
                      .transpose(4, 1, 2, 3, 0).reshape(BLOC, L, 4 * H))
    return outs, states


_CACHE = {}


def _get_runner(L, wsteps, body_windows, use_for_i):
    key = (L, wsteps, body_windows, use_for_i)
    if key in _CACHE:
        return _CACHE[key]
    import jax
    from jax.sharding import Mesh, PartitionSpec
    from jax.experimental.shard_map import shard_map
    from concourse.bass2jax import (_bass_exec_p, install_neuronx_cc_hook,
                                    partition_id_tensor)
    nc = build_nc(L, wsteps=wsteps, body_windows=body_windows,
                  use_for_i=use_for_i)
    install_neuronx_cc_hook()
    partition_name = (nc.partition_id_tensor.name
                      if nc.partition_id_tensor else None)
    in_names, out_names, out_avals, zero_shapes = [], [], [], []
    for alloc in nc.m.functions[0].allocations:
        if not isinstance(alloc, mybir.MemoryLocationSet):
            continue
        name = alloc.memorylocations[0].name
        if alloc.kind == "ExternalInput":
            if name != partition_name:
                in_names.append(name)
        elif alloc.kind == "ExternalOutput":
            shape = tuple(alloc.tensor_shape)
            dtype = mybir.dt.np(alloc.dtype)
            out_names.append(name)
            out_avals.append(jax.core.ShapedArray(shape, dtype))
            zero_shapes.append((shape, dtype))
    n_params = len(in_names)
    all_in = in_names + out_names + ([partition_name] if partition_name else [])

    def _body(*args):
        operands = list(args)
        if partition_name is not None:
            operands.append(partition_id_tensor())
        outs = _bass_exec_p.bind(
            *operands, out_avals=tuple(out_avals), in_names=tuple(all_in),
            out_names=tuple(out_names), lowering_input_output_aliases=(),
            sim_require_finite=True, sim_require_nnan=True, nc=nc)
        return tuple(outs)

    devices = jax.devices()[:NCORES]
    mesh = Mesh(np.asarray(devices), ("core",))
    n_outs = len(out_avals)
    sharded = jax.jit(
        shard_map(_body, mesh=mesh,
                  in_specs=(PartitionSpec("core"),) * (n_params + n_outs),
                  out_specs=(PartitionSpec("core"),) * n_outs,
                  check_rep=False),
        keep_unused=True)

    def run(in_maps):
        import jax as _jax
        concat_in = [np.concatenate([np.asarray(in_maps[c][n])
                                     for c in range(NCORES)], axis=0)
                     for n in in_names]
        concat_zeros = [np.zeros((NCORES * s[0],) + tuple(s[1:]), d)
                        for (s, d) in zero_shapes]
        out = sharded(*concat_in, *concat_zeros)
        _jax.block_until_ready(out)
        return [{name: np.asarray(out[i]).reshape((NCORES,)
                                                  + out_avals[i].shape)[c]
                 for i, name in enumerate(out_names)}
                for c in range(NCORES)]

    _CACHE[key] = run
    return run


def kernel(x, time_deltas, start, weight, bias,
           L=None, wsteps=64, body_windows=2, use_for_i=True):
    x = np.asarray(x)
    L = x.shape[1] if L is None else L
    run = _get_runner(L, wsteps, body_windows, use_for_i)
    ins = _prep_inputs(x, time_deltas, start, weight, bias, L, wsteps)
    results = run(ins)
    return _gather(results, L)
